# revision 1
# baseline (speedup 1.0000x reference)
"""Weighted 2D cross-entropy (BCE-over-classes) loss on 8 Trainium2 cores.

Math (matches the reference):
  t in [0,19); pos = t>0, neg = t==0 (all pixels are pos or neg; mask == 1)
  S(i) = sum_c bce(i,c) = -lnR(i)
     lnR(i) = A(i) + B(i)
     A(i)   = sum_c ln(1-p_c(i))
     B(i)   = ln(p_t(i)) - ln(1-p_t(i)) = ln(e^{-lsel(i)} - 1),  lsel = ln(1-p_t)
  loss = ( (NEG/TOT)*S_pos_sum + (POS/TOT)*S_neg_sum ) / (TOT*C)

Per-core (core k <- batch element k, pure data parallel), two mega-passes
over pixel halves [128, 2048] (PSUM holds A + lsel for one half = 8 banks):
  - per class: 1MB DMA of p, ACT Ln(1-p)->bf16, DVE eq=(t==c) (4x) and
    masked=eq*L (2x), PE identity-matmuls accumulate A and lsel in PSUM.
  - tail per half, 2 chunks of 1024 (pipelines ACT against DVE):
    expm=Exp(-lsel); em1=expm-1 (DVE); B=Ln(em1); lnR=B+A via STT with
    accum_out; pos-masked sum via a second STT accum.
  - last class of the last pass is processed in two 1024 chunks so the
    final tail starts sooner after the last DMA byte.
Activation tables are pinned to natural_log_exp_and_others (holds both
ln and exp) -- otherwise bacc's table-load pass alternates between the
ln-only and exp-only sets, paying ~1.3us per reload 5x per kernel.
Counts (pos/neg) are computed on host from the int target directly.
Per-core output is the raw [128, 8] per-partition stats; the final
partition reduce + 8-way combine happens on host in float64.
"""

from contextlib import ExitStack

import numpy as np

import concourse.bass as bass
import concourse.mybir as mybir
import concourse.tile as tile
from concourse import bacc
from concourse.bass_utils import run_bass_kernel_spmd

# problem shape (hardcoded per harness contract)
N, C, H, W = 8, 19, 512, 1024
PIX = H * W          # 524288 pixels per core
P = 128              # partitions
FCOLS = PIX // P     # 4096 free columns when pixels laid out [128, 4096]
HWID = FCOLS // 2    # 2048: half-width processed per mega-pass
QW = HWID // 2       # 1024: tail chunk width
N_CORES = 8
NSTAT = 8            # stats columns in the [128, 8] output

DT = mybir.dt

# stats column layout ([128, 8] f32; host folds):
#   0-3: sum lnR      for (half, chunk) = col h*2+ch
#   4-7: sum pos*lnR  for (half, chunk) = col 4 + h*2+ch
COL_LNR = 0
COL_POSLNR = 4

_ACT_TABLES_PATCHED = False


def _pin_act_table_set():
    """Restrict Ln/Exp to the natural_log_exp_and_others set so bacc's
    table-load pass emits a single ACT_TABLE_LOAD instead of thrashing
    between the ln-only and exp-only sets (~1.3us per reload).  Set
    indices must stay aligned with act_info.json, so every set entry is
    kept -- only the Ln/Exp membership of the other sets is dropped."""
    global _ACT_TABLES_PATCHED
    if _ACT_TABLES_PATCHED:
        return
    import concourse.bacc as bacc_mod

    orig = bacc_mod.get_activation_tables
    ln_exp = {mybir.ActivationFunctionType.Ln, mybir.ActivationFunctionType.Exp}

    def patched(arch):
        tables = orig(arch)
        return {
            name: (fns if name == "natural_log_exp_and_others" else fns - ln_exp)
            for name, fns in tables.items()
        }

    bacc_mod.get_activation_tables = patched
    _ACT_TABLES_PATCHED = True


def build_kernel() -> bass.Bass:
    _pin_act_table_set()

    # Bacc (not raw Bass): its compile() pipeline runs
    # generate_event_semaphores, which splits multi-sem waits to satisfy the
    # 1-wait-per-instruction TRN2 sync structs -- raw Bass modules with
    # Tile-emitted multi-waits fail walrus codegen.
    nc = bacc.Bacc("TRN2")

    predict = nc.declare_dram_parameter("predict", [C, PIX], DT.float32, isOutput=False)
    target = nc.declare_dram_parameter("target", [P, FCOLS], DT.int32, isOutput=False)
    idn = nc.declare_dram_parameter("idn", [P, P], DT.bfloat16, isOutput=False)
    out = nc.declare_dram_parameter("out", [P, NSTAT], DT.float32, isOutput=True)

    pred_r = predict.rearrange("c (p f) -> c p f", p=P)  # [19, 128, 4096]

    with tile.TileContext(nc) as tc, ExitStack() as ctx:
        const = ctx.enter_context(tc.tile_pool(name="const", bufs=1))
        t_pool = ctx.enter_context(tc.tile_pool(name="t", bufs=2))
        # p bufs=8 aligns slot reuse with the global DMA->DMAHW-proc
        # round-robin (8 procs), so the WAW on the old writer is same-proc
        # FIFO order and Tile emits no cross-queue wait
        p_pool = ctx.enter_context(tc.tile_pool(name="p", bufs=8))
        lm_pool = ctx.enter_context(tc.tile_pool(name="lm", bufs=4))
        eq_pool = ctx.enter_context(tc.tile_pool(name="eq", bufs=4))
        tail_pool = ctx.enter_context(tc.tile_pool(name="tail", bufs=2))
        psA_pool = ctx.enter_context(tc.tile_pool(name="psA", bufs=1, space="PSUM"))
        psL_pool = ctx.enter_context(tc.tile_pool(name="psL", bufs=1, space="PSUM"))

        idn_sb = const.tile([P, P], DT.bfloat16, tag="idn")
        nc.sync.dma_start(out=idn_sb[:], in_=idn[:])

        stats = const.tile([P, NSTAT], DT.float32, tag="stats")
        nc.vector.memset(stats[:], 0.0)

        t_bf = const.tile([P, FCOLS], DT.bfloat16, tag="tb")

        for h in range(2):
            fsl = slice(h * HWID, (h + 1) * HWID)
            t_i32 = t_pool.tile([P, HWID], DT.int32, tag="ti")
            nc.sync.dma_start(out=t_i32[:], in_=target[:, fsl])
            nc.vector.tensor_copy(out=t_bf[:, fsl], in_=t_i32[:])

        for h in range(2):
            fsl = slice(h * HWID, (h + 1) * HWID)
            t_sl = t_bf[:, fsl]

            # PSUM accumulators for this half: A = sum_c L_c, lsel = L_t
            a_ps = psA_pool.tile([P, HWID], DT.float32, tag="aps")
            l_ps = psL_pool.tile([P, HWID], DT.float32, tag="lps")

            for c in range(C):
                # the last class of the last pass is split into two 1024
                # chunks so the tail can start on chunk 0 while chunk 1
                # still computes
                if h == 1 and c == C - 1:
                    chunks = [(q * QW, QW) for q in range(2)]
                else:
                    chunks = [(0, HWID)]

                for off, width in chunks:
                    csl = slice(h * HWID + off, h * HWID + off + width)
                    p_t = p_pool.tile([P, width], DT.float32, tag="p")
                    nc.sync.dma_start(out=p_t[:, :width], in_=pred_r[c, :, csl])

                    # lm[:, :w] = L_c = Ln(1-p) bf16 ; lm[:, w:] = (T==c)*L_c
                    lm = lm_pool.tile([P, 2 * HWID], DT.bfloat16, tag="lm")
                    nc.scalar.activation(
                        out=lm[:, :width],
                        in_=p_t[:, :width],
                        func=mybir.ActivationFunctionType.Ln,
                        bias=1.0,
                        scale=-1.0,
                    )
                    # eq at DVE 4x (16-bit tensor_scalar) + mult at 2x beats
                    # the fused scalar_tensor_tensor, which only has a 1x uop
                    eq = eq_pool.tile([P, HWID], DT.bfloat16, tag="eq")
                    nc.vector.tensor_scalar(
                        out=eq[:, :width],
                        in0=t_bf[:, csl],
                        scalar1=float(c),
                        scalar2=None,
                        op0=mybir.AluOpType.is_equal,
                    )
                    nc.vector.tensor_mul(
                        out=lm[:, HWID : HWID + width],
                        in0=eq[:, :width],
                        in1=lm[:, :width],
                    )

                    # lsel matmuls first: l_ps frees early in the tail (Exp
                    # is its only reader), so the next pass's PE work
                    # restarts sooner
                    for s in range(width // 512):
                        src = slice(HWID + s * 512, HWID + (s + 1) * 512)
                        dst = slice(off + s * 512, off + (s + 1) * 512)
                        nc.tensor.matmul(
                            l_ps[:, dst],
                            lhsT=idn_sb[:],
                            rhs=lm[:, src],
                            start=(c == 0),
                            stop=(c == C - 1),
                        )
                    for s in range(width // 512):
                        src = slice(s * 512, (s + 1) * 512)
                        dst = slice(off + s * 512, off + (s + 1) * 512)
                        nc.tensor.matmul(
                            a_ps[:, dst],
                            lhsT=idn_sb[:],
                            rhs=lm[:, src],
                            start=(c == 0),
                            stop=(c == C - 1),
                        )

            # tail, 2 chunks of 1024: B = Ln(e^{-lsel} - 1); lnR = B + A
            for ch in range(2):
                qsl = slice(ch * QW, (ch + 1) * QW)
                col = h * 2 + ch
                expm = tail_pool.tile([P, QW], DT.float32, tag="expm")
                nc.scalar.activation(
                    out=expm[:],
                    in_=l_ps[:, qsl],
                    func=mybir.ActivationFunctionType.Exp,
                    scale=-1.0,
                )
                em1 = tail_pool.tile([P, QW], DT.float32, tag="em1")
                nc.vector.tensor_scalar(
                    out=em1[:],
                    in0=expm[:],
                    scalar1=1.0,
                    scalar2=None,
                    op0=mybir.AluOpType.subtract,
                )
                bb = tail_pool.tile([P, QW], DT.float32, tag="bb")
                nc.scalar.activation(
                    out=bb[:],
                    in_=em1[:],
                    func=mybir.ActivationFunctionType.Ln,
                )
                lnr = tail_pool.tile([P, QW], DT.float32, tag="lnr")
                nc.vector.scalar_tensor_tensor(
                    out=lnr[:],
                    in0=bb[:],
                    scalar=0.0,
                    in1=a_ps[:, qsl],
                    op0=mybir.AluOpType.add,
                    op1=mybir.AluOpType.add,
                    accum_out=stats[:, COL_LNR + col : COL_LNR + col + 1],
                )
                scr = tail_pool.tile([P, QW], DT.float32, tag="scr")
                nc.vector.scalar_tensor_tensor(
                    out=scr[:],
                    in0=t_bf[:, h * HWID + ch * QW : h * HWID + (ch + 1) * QW],
                    scalar=0.5,
                    in1=lnr[:],
                    op0=mybir.AluOpType.is_gt,
                    op1=mybir.AluOpType.mult,
                    accum_out=stats[:, COL_POSLNR + col : COL_POSLNR + col + 1],
                )

        nc.sync.dma_start(out=out[:], in_=stats[:])

    if not nc.is_finalized():
        nc.finalize()

    return nc


_NC_CACHE = None


def make_in_maps(predict: np.ndarray, target: np.ndarray):
    import ml_dtypes

    predict = np.ascontiguousarray(predict, dtype=np.float32)
    target = np.ascontiguousarray(target, dtype=np.int32)
    idn = np.eye(P, dtype=np.float32).astype(ml_dtypes.bfloat16)

    in_maps = []
    for k in range(N_CORES):
        in_maps.append(
            {
                "predict": predict[k].reshape(C, PIX),
                "target": target[k].reshape(P, FCOLS),
                "idn": idn,
            }
        )
    return in_maps


def combine_host(results, target: np.ndarray) -> np.float32:
    tot = np.float64(0.0)
    s_all = np.float64(0.0)
    s_pos = np.float64(0.0)
    for k in range(N_CORES):
        st = results[k]["out"].reshape(P, NSTAT).astype(np.float64)
        s_all += -np.sum(st[:, COL_LNR : COL_LNR + 4])
        s_pos += -np.sum(st[:, COL_POSLNR : COL_POSLNR + 4])
        tot += PIX
    pos = np.float64(np.count_nonzero(target))
    neg = tot - pos
    s_neg = s_all - s_pos
    loss = ((neg / tot) * s_pos + (pos / tot) * s_neg) / (tot * C)
    return np.float32(loss)


def kernel(predict: np.ndarray, target: np.ndarray) -> np.ndarray:
    global _NC_CACHE
    if _NC_CACHE is None:
        _NC_CACHE = build_kernel()
    nc = _NC_CACHE

    in_maps = make_in_maps(predict, target)
    res = run_bass_kernel_spmd(nc, in_maps, list(range(N_CORES)))
    return combine_host(res.results, target)



# revision 5
# speedup vs baseline: 1.0382x; 1.0382x over previous
"""Weighted 2D cross-entropy (BCE-over-classes) loss on 8 Trainium2 cores.

Math (matches the reference):
  t in [0,19); pos = t>0, neg = t==0 (all pixels are pos or neg; mask == 1)
  S(i) = sum_c bce(i,c) = -lnR(i)
     lnR(i) = A(i) + B(i)
     A(i)   = sum_c ln(1-p_c(i))
     B(i)   = ln(p_t(i)) - ln(1-p_t(i)) = ln(e^{-lsel(i)} - 1),  lsel = ln(1-p_t)
  loss = ( (NEG/TOT)*S_pos_sum + (POS/TOT)*S_neg_sum ) / (TOT*C)

Per-core (core k <- batch element k, pure data parallel), FOUR quarter-passes
over pixel quarters [128, 1024].  A quarter's PSUM accumulators (A + lsel)
occupy 4 banks, so two quarters ping-pong in PSUM: while quarter q's tail
(Exp/Ln/STT chain on ACT+DVE) drains its PSUM banks, the PE already streams
quarter q+1's matmuls -- this removes the half-boundary pipeline stall the
previous 2-half version paid (~6-8us PE idle per boundary).
  - per class: 0.5MB DMA of p, ACT Ln(1-p)->bf16, DVE eq=(t==c) (4x) and
    masked=eq*L (2x), PE identity-matmuls accumulate A and lsel in PSUM.
  - tail per quarter: expm=Exp(-lsel); B=Ln(expm-1) (fused bias, no DVE
    subtract); lnR=B+A via STT with accum_out; pos-masked sum via a second
    STT accum.
  - last class of the last quarter and the final tail run in 512-wide
    chunks so the post-last-DMA drain is short.
Target is converted to bf16 on HOST (1MB instead of 2MB int32 DMA, no
on-chip CAST, and the first predict tile lands sooner).
Activation tables are pinned to natural_log_exp_and_others (holds both
ln and exp) -- otherwise bacc's table-load pass alternates between the
ln-only and exp-only sets, paying ~1.3us per reload.
Counts (pos/neg) are computed on host from the int target directly.
Per-core output is the raw [128, 16] per-partition stats; the final
partition reduce + 8-way combine happens on host in float64.
"""

from contextlib import ExitStack

import numpy as np

import concourse.bass as bass
import concourse.mybir as mybir
import concourse.tile as tile
from concourse import bacc
from concourse.bass_utils import run_bass_kernel_spmd

# problem shape (hardcoded per harness contract)
N, C, H, W = 8, 19, 512, 1024
PIX = H * W          # 524288 pixels per core
P = 128              # partitions
FCOLS = PIX // P     # 4096 free columns when pixels laid out [128, 4096]
QW = FCOLS // 4      # 1024: quarter width
HQW = QW // 2        # 512: final-chunk width
N_CORES = 8
NSTAT = 16           # stats columns in the [128, 16] output

DT = mybir.dt

# stats column layout ([128, 16] f32; host folds):
#   0-2 : sum lnR      for quarters 0-2
#   3-4 : sum lnR      for quarter 3 chunks 0,1
#   8-10: sum pos*lnR  for quarters 0-2
#   11-12: sum pos*lnR for quarter 3 chunks 0,1
COL_LNR = 0
COL_POSLNR = 8

_ACT_TABLES_PATCHED = False


def _pin_act_table_set():
    """Restrict Ln/Exp to the natural_log_exp_and_others set so bacc's
    table-load pass emits a single ACT_TABLE_LOAD instead of thrashing
    between the ln-only and exp-only sets (~1.3us per reload).  Set
    indices must stay aligned with act_info.json, so every set entry is
    kept -- only the Ln/Exp membership of the other sets is dropped."""
    global _ACT_TABLES_PATCHED
    if _ACT_TABLES_PATCHED:
        return
    import concourse.bacc as bacc_mod

    orig = bacc_mod.get_activation_tables
    ln_exp = {mybir.ActivationFunctionType.Ln, mybir.ActivationFunctionType.Exp}

    def patched(arch):
        tables = orig(arch)
        return {
            name: (fns if name == "natural_log_exp_and_others" else fns - ln_exp)
            for name, fns in tables.items()
        }

    bacc_mod.get_activation_tables = patched
    _ACT_TABLES_PATCHED = True


def build_kernel() -> bass.Bass:
    _pin_act_table_set()

    # Bacc (not raw Bass): its compile() pipeline runs
    # generate_event_semaphores, which splits multi-sem waits to satisfy the
    # 1-wait-per-instruction TRN2 sync structs -- raw Bass modules with
    # Tile-emitted multi-waits fail walrus codegen.
    nc = bacc.Bacc("TRN2")

    predict = nc.declare_dram_parameter("predict", [C, PIX], DT.float32, isOutput=False)
    target = nc.declare_dram_parameter("target", [P, FCOLS], DT.bfloat16, isOutput=False)
    idn = nc.declare_dram_parameter("idn", [P, P], DT.bfloat16, isOutput=False)
    out = nc.declare_dram_parameter("out", [P, NSTAT], DT.float32, isOutput=True)

    pred_r = predict.rearrange("c (p f) -> c p f", p=P)  # [19, 128, 4096]

    with tile.TileContext(nc) as tc, ExitStack() as ctx:
        const = ctx.enter_context(tc.tile_pool(name="const", bufs=1))
        # p bufs=16 (multiple of the 8 DMA procs: slot reuse stays same-proc
        # FIFO order, no cross-queue wait) = 8MB SBUF = ~19us of DMA lookahead
        p_pool = ctx.enter_context(tc.tile_pool(name="p", bufs=16))
        lm_pool = ctx.enter_context(tc.tile_pool(name="lm", bufs=4))
        eq_pool = ctx.enter_context(tc.tile_pool(name="eq", bufs=4))
        tail_pool = ctx.enter_context(tc.tile_pool(name="tail", bufs=2))
        psA_pool = ctx.enter_context(tc.tile_pool(name="psA", bufs=2, space="PSUM"))
        psL_pool = ctx.enter_context(tc.tile_pool(name="psL", bufs=2, space="PSUM"))

        t_bf = const.tile([P, FCOLS], DT.bfloat16, tag="tb")
        # quarter 0 of target first so the q0 eq chain is ready before p0
        nc.sync.dma_start(out=t_bf[:, 0:QW], in_=target[:, 0:QW])

        idn_sb = const.tile([P, P], DT.bfloat16, tag="idn")
        stats = const.tile([P, NSTAT], DT.float32, tag="stats")
        # per-partition -1.0 bias column for the fused Ln(expm - 1) tail
        negone = const.tile([P, 1], DT.float32, tag="negone")

        first_issued = False

        for q in range(4):
            qbase = q * QW
            # PSUM accumulators for this quarter (ping-pong, 2+2 banks each)
            a_ps = psA_pool.tile([P, QW], DT.float32, tag="aps")
            l_ps = psL_pool.tile([P, QW], DT.float32, tag="lps")

            for c in range(C):
                # the last class of the last quarter is split into two 512
                # chunks so the tail can start on chunk 0 while chunk 1
                # still computes
                if q == 3 and c == C - 1:
                    chunks = [(s * HQW, HQW) for s in range(2)]
                else:
                    chunks = [(0, QW)]

                for off, width in chunks:
                    csl = slice(qbase + off, qbase + off + width)
                    p_t = p_pool.tile([P, width], DT.float32, tag="p")
                    nc.sync.dma_start(out=p_t[:, :width], in_=pred_r[c, :, csl])

                    if not first_issued:
                        # small constants + remaining target quarters queue
                        # behind p(q0,c0) so the pipeline primes first
                        first_issued = True
                        nc.sync.dma_start(out=idn_sb[:], in_=idn[:])
                        nc.vector.memset(stats[:], 0.0)
                        nc.vector.memset(negone[:], -1.0)
                        nc.sync.dma_start(out=t_bf[:, QW:], in_=target[:, QW:])

                    # lm[:, :w] = L_c = Ln(1-p) bf16 ; lm[:, w:] = (T==c)*L_c
                    lm = lm_pool.tile([P, 2 * QW], DT.bfloat16, tag="lm")
                    nc.scalar.activation(
                        out=lm[:, :width],
                        in_=p_t[:, :width],
                        func=mybir.ActivationFunctionType.Ln,
                        bias=1.0,
                        scale=-1.0,
                    )
                    # eq at DVE 4x (16-bit tensor_scalar) + mult at 2x beats
                    # the fused scalar_tensor_tensor, which only has a 1x uop
                    eq = eq_pool.tile([P, QW], DT.bfloat16, tag="eq")
                    nc.vector.tensor_scalar(
                        out=eq[:, :width],
                        in0=t_bf[:, csl],
                        scalar1=float(c),
                        scalar2=None,
                        op0=mybir.AluOpType.is_equal,
                    )
                    nc.vector.tensor_mul(
                        out=lm[:, QW : QW + width],
                        in0=eq[:, :width],
                        in1=lm[:, :width],
                    )

                    # lsel matmuls first: l_ps frees early in the tail (Exp
                    # is its only reader), so the next quarter's PE work
                    # restarts sooner
                    for s in range(width // HQW):
                        src = slice(QW + s * HQW, QW + (s + 1) * HQW)
                        dst = slice(off + s * HQW, off + (s + 1) * HQW)
                        nc.tensor.matmul(
                            l_ps[:, dst],
                            lhsT=idn_sb[:],
                            rhs=lm[:, src],
                            start=(c == 0),
                            stop=(c == C - 1),
                        )
                    for s in range(width // HQW):
                        src = slice(s * HQW, (s + 1) * HQW)
                        dst = slice(off + s * HQW, off + (s + 1) * HQW)
                        nc.tensor.matmul(
                            a_ps[:, dst],
                            lhsT=idn_sb[:],
                            rhs=lm[:, src],
                            start=(c == 0),
                            stop=(c == C - 1),
                        )

            # tail: B = Ln(e^{-lsel} - 1) (bias fuses the -1); lnR = B + A.
            # quarter 3 drains in two 512 chunks to shorten the final latency
            # chain after the last DMA byte.
            tail_chunks = [(s * HQW, HQW) for s in range(2)] if q == 3 else [(0, QW)]
            for ci, (toff, twidth) in enumerate(tail_chunks):
                qsl = slice(toff, toff + twidth)
                col = q + ci if q < 3 else 3 + ci
                expm = tail_pool.tile([P, QW], DT.float32, tag="expm")
                nc.scalar.activation(
                    out=expm[:, :twidth],
                    in_=l_ps[:, qsl],
                    func=mybir.ActivationFunctionType.Exp,
                    scale=-1.0,
                )
                bb = tail_pool.tile([P, QW], DT.float32, tag="bb")
                nc.scalar.activation(
                    out=bb[:, :twidth],
                    in_=expm[:, :twidth],
                    func=mybir.ActivationFunctionType.Ln,
                    bias=negone[:],
                )
                lnr = tail_pool.tile([P, QW], DT.float32, tag="lnr")
                nc.vector.scalar_tensor_tensor(
                    out=lnr[:, :twidth],
                    in0=bb[:, :twidth],
                    scalar=0.0,
                    in1=a_ps[:, qsl],
                    op0=mybir.AluOpType.add,
                    op1=mybir.AluOpType.add,
                    accum_out=stats[:, COL_LNR + col : COL_LNR + col + 1],
                )
                scr = tail_pool.tile([P, QW], DT.float32, tag="scr")
                nc.vector.scalar_tensor_tensor(
                    out=scr[:, :twidth],
                    in0=t_bf[:, qbase + toff : qbase + toff + twidth],
                    scalar=0.5,
                    in1=lnr[:, :twidth],
                    op0=mybir.AluOpType.is_gt,
                    op1=mybir.AluOpType.mult,
                    accum_out=stats[:, COL_POSLNR + col : COL_POSLNR + col + 1],
                )

        nc.sync.dma_start(out=out[:], in_=stats[:])

    if not nc.is_finalized():
        nc.finalize()

    return nc


_NC_CACHE = None


def make_in_maps(predict: np.ndarray, target: np.ndarray):
    import ml_dtypes

    predict = np.ascontiguousarray(predict, dtype=np.float32)
    target_bf = np.ascontiguousarray(target, dtype=np.int32).astype(ml_dtypes.bfloat16)
    idn = np.eye(P, dtype=np.float32).astype(ml_dtypes.bfloat16)

    in_maps = []
    for k in range(N_CORES):
        in_maps.append(
            {
                "predict": predict[k].reshape(C, PIX),
                "target": target_bf[k].reshape(P, FCOLS),
                "idn": idn,
            }
        )
    return in_maps


def combine_host(results, target: np.ndarray) -> np.float32:
    tot = np.float64(0.0)
    s_all = np.float64(0.0)
    s_pos = np.float64(0.0)
    for k in range(N_CORES):
        st = results[k]["out"].reshape(P, NSTAT).astype(np.float64)
        s_all += -np.sum(st[:, COL_LNR : COL_LNR + 5])
        s_pos += -np.sum(st[:, COL_POSLNR : COL_POSLNR + 5])
        tot += PIX
    pos = np.float64(np.count_nonzero(target))
    neg = tot - pos
    s_neg = s_all - s_pos
    loss = ((neg / tot) * s_pos + (pos / tot) * s_neg) / (tot * C)
    return np.float32(loss)


def kernel(predict: np.ndarray, target: np.ndarray) -> np.ndarray:
    global _NC_CACHE
    if _NC_CACHE is None:
        _NC_CACHE = build_kernel()
    nc = _NC_CACHE

    in_maps = make_in_maps(predict, target)
    res = run_bass_kernel_spmd(nc, in_maps, list(range(N_CORES)))
    return combine_host(res.results, target)
